# revision 11
# baseline (speedup 1.0000x reference)
"""AnchorSegmentMixer Trainium2 kernel (8 NeuronCores, batch-sharded).

reference:
    energy[n] = mean(w[n]**2)                       # [B]
    ratio[n]  = clip(sqrt(energy[n]/max(energy[n+1 mod B], 1e-10)), 0.02, 50)
    mixtures  = w + ratio[:, None] * roll(w, -1, axis=0)
    returns (mixtures, targets=w)

Sharding: pure data parallel over the batch axis. Core c receives rows
[32c, 32c+32] (33 rows: 32 output rows + 1 circular halo row), computes all 33
row energies locally, and emits its 32 mixture rows. No collectives needed.

On-chip layout: each 160000-sample row is spread over the 128 SBUF partitions
as [128, 1250] (partition p holds samples [1250p, 1250(p+1))), and the whole
33-row shard stays resident in SBUF (161 KiB/partition) so HBM traffic is the
roofline minimum: read 33 rows + write 32 rows per core.

Structure: the 32 output rows are processed as 4 blocks of 8. Each block's
ratios only need energies of rows [8k, 8k+8], so block k's store phase
overlaps block k+1's load phase and the DMA engines stay saturated.

Engine split (measured per-[128,1250]-op costs): ACT does the 33 energy
squares (activation+accum_out, ~1.6us each) during the load phase, GpSimd the
32 ratio-scale multiplies (~1.5us), DVE the 32 adds (~1.75us) during the
store phase. vector.tensor_tensor_reduce is avoided - it crashes this runtime.
"""

import numpy as np

B = 256
S = 160000
P = 128
F = S // P            # 1250 samples per partition per row
N_CORES = 8
OUT_ROWS = B // N_CORES   # 32
ROWS = OUT_ROWS + 1       # +1 halo row
EPS = 1e-10
INV_N = 1.0 / S

# pipelined block sizes: small first block (fast ramp to the first output
# DMAs), small last block (short drain tail), 8-row blocks in the middle
BLOCK_SIZES = (4, 8, 8, 8, 4)
assert sum(BLOCK_SIZES) == OUT_ROWS

_cache = {}


def _build_nc():
    from contextlib import ExitStack

    import concourse.bass as bass
    import concourse.tile as tile
    from concourse import bacc, mybir

    nc = bacc.Bacc("TRN2", target_bir_lowering=False, debug=False,
                   num_devices=N_CORES)
    f32 = mybir.dt.float32
    wv = nc.declare_dram_parameter("waveforms", [ROWS, S], f32, isOutput=False)
    out = nc.declare_dram_parameter("out", [OUT_ROWS, S], f32, isOutput=True)

    in_v = wv.ap().rearrange("r (p f) -> p r f", p=P)    # [128, 33, 1250]
    out_v = out.ap().rearrange("r (p f) -> p r f", p=P)  # [128, 32, 1250]

    with tile.TileContext(nc) as tc, ExitStack() as ctx:
        data_pool = ctx.enter_context(tc.tile_pool(name="data", bufs=1))
        scr_pool = ctx.enter_context(tc.tile_pool(name="scr", bufs=1))
        outp = ctx.enter_context(tc.tile_pool(name="outp", bufs=3))
        singles = ctx.enter_context(tc.tile_pool(name="singles", bufs=1))
        psum = ctx.enter_context(tc.tile_pool(name="psum", bufs=2, space="PSUM"))

        data = data_pool.tile([P, ROWS * F], f32)
        partials = singles.tile([P, ROWS], f32)       # per-partition sum(x^2)
        inv_n_col = singles.tile([P, 1], f32)         # 1/S for the mean matmul
        ones_row = singles.tile([1, P], f32)          # broadcast matmul lhsT
        e_sb = singles.tile([1, ROWS], f32)           # mean energies
        e_bc = singles.tile([P, ROWS], f32)           # energies on all partitions
        denom = singles.tile([P, OUT_ROWS], f32)
        ratio = singles.tile([P, OUT_ROWS], f32)      # clipped mix ratios
        sq_act = scr_pool.tile([P, F], f32, tag="sq_act")

        nc.vector.memset(inv_n_col[:], INV_N)
        nc.gpsimd.memset(ones_row[:], 1.0)

        def load_rows(r0, r1, split=1):
            # in-DMAs ride GpSimd/SWDGE: gpsimd is otherwise idle, so loads
            # are never queued behind out-DMAs on Sync's in-order stream
            step = max(1, (r1 - r0 + split - 1) // split)
            for g in range(r0, r1, step):
                ge = min(g + step, r1)
                nc.gpsimd.dma_start(out=data[:, g * F:ge * F],
                                    in_=in_v[:, g:ge, :])

        def square(r):
            nc.scalar.activation(
                out=sq_act[:], in_=data[:, r * F:(r + 1) * F],
                func=mybir.ActivationFunctionType.Square,
                accum_out=partials[:, r:r + 1],
            )

        def block_ratio(lo, hi):
            # energies for rows [lo, hi] -> ratio[:, lo:hi] on all partitions
            n = hi - lo + 1
            e_ps = psum.tile([1, n], f32, tag="e")
            nc.tensor.matmul(e_ps[:], inv_n_col[:], partials[:, lo:hi + 1],
                             start=True, stop=True)
            nc.vector.tensor_copy(e_sb[:, lo:hi + 1], e_ps[:])
            bc_ps = psum.tile([P, n], f32, tag="bc")
            nc.tensor.matmul(bc_ps[:], ones_row[:], e_sb[:, lo:hi + 1],
                             start=True, stop=True)
            nc.vector.tensor_copy(e_bc[:, lo:hi + 1], bc_ps[:])
            rs = slice(lo, hi)
            nc.vector.tensor_scalar_max(denom[:, rs], e_bc[:, lo + 1:hi + 1], EPS)
            nc.vector.reciprocal(denom[:, rs], denom[:, rs])
            nc.vector.tensor_mul(denom[:, rs], e_bc[:, lo:hi], denom[:, rs])
            nc.scalar.sqrt(ratio[:, rs], denom[:, rs])
            nc.vector.tensor_scalar(
                out=ratio[:, rs], in0=ratio[:, rs], scalar1=50.0, scalar2=0.02,
                op0=mybir.AluOpType.min, op1=mybir.AluOpType.max,
            )

        def mix_row(r, on_dve):
            # out[r] = w[r] + ratio[r] * w[r+1]; adds on DVE, scales split
            # ACT/DVE to balance engine busy (ACT also owns the squares).
            # (NOT gpsimd: its tensor_scalar measured 19us/op vs ACT 1.6us)
            o = outp.tile([P, F], f32, tag="o")
            nxt = data[:, (r + 1) * F:(r + 2) * F]
            if on_dve:
                nc.vector.tensor_scalar_mul(o[:], nxt, ratio[:, r:r + 1])
            else:
                nc.scalar.mul(o[:], nxt, mul=ratio[:, r:r + 1])
            nc.vector.tensor_add(o[:], o[:], data[:, r * F:(r + 1) * F])
            nc.sync.dma_start(out=out_v[:, r, :], in_=o[:])

        # Software pipeline over blocks, ratio computed TWO blocks ahead of
        # its use so the ~5us matmul+chain latency never gates the store
        # phase. Emission order also keeps block k's scale-muls before block
        # k+2's squares on ACT, so out-DMAs are never starved behind a
        # square batch.
        nb = len(BLOCK_SIZES)
        starts = [sum(BLOCK_SIZES[:i]) for i in range(nb + 1)]

        def load_sq_ratio(k, split=1):
            # load block k's not-yet-loaded rows (incl. its halo = first row
            # of block k+1), square them, then block k's ratio columns
            lo = starts[k] + (1 if k else 0)
            hi = starts[k + 1] + 1
            load_rows(lo, hi, split=split)
            for r in range(lo, hi):
                square(r)
            block_ratio(starts[k], starts[k + 1])

        load_sq_ratio(0, split=BLOCK_SIZES[0] + 1)   # per-row DMAs: fast ramp
        load_sq_ratio(1)
        for k in range(nb):
            for i, r in enumerate(range(starts[k], starts[k + 1])):
                mix_row(r, on_dve=(i % 8 < 3))
            if k + 2 < nb:
                load_sq_ratio(k + 2)

    nc.compile()
    return nc


def _get_nc():
    if "nc" not in _cache:
        _cache["nc"] = _build_nc()
    return _cache["nc"]


def _shard_inputs(waveforms):
    in_maps = []
    for c in range(N_CORES):
        rows = (np.arange(c * OUT_ROWS, c * OUT_ROWS + ROWS)) % B
        in_maps.append({"waveforms": np.ascontiguousarray(waveforms[rows])})
    return in_maps


def kernel(waveforms):
    from concourse.bass_utils import run_bass_kernel_spmd

    waveforms = np.asarray(waveforms, dtype=np.float32)
    nc = _get_nc()
    in_maps = _shard_inputs(waveforms)
    res = run_bass_kernel_spmd(nc, in_maps, list(range(N_CORES)))
    mixtures = np.concatenate(
        [res.results[c]["out"] for c in range(N_CORES)], axis=0
    )
    return mixtures, waveforms


# revision 13
# speedup vs baseline: 1.0356x; 1.0356x over previous
"""AnchorSegmentMixer Trainium2 kernel (8 NeuronCores, batch-sharded).

reference:
    energy[n] = mean(w[n]**2)                       # [B]
    ratio[n]  = clip(sqrt(energy[n]/max(energy[n+1 mod B], 1e-10)), 0.02, 50)
    mixtures  = w + ratio[:, None] * roll(w, -1, axis=0)
    returns (mixtures, targets=w)

Sharding: pure data parallel over the batch axis. Core c receives rows
[32c, 32c+32] (33 rows: 32 output rows + 1 circular halo row), computes all 33
row energies locally, and emits its 32 mixture rows. No collectives needed.

On-chip layout: each 160000-sample row is spread over the 128 SBUF partitions
as [128, 1250] (partition p holds samples [1250p, 1250(p+1))), and the whole
33-row shard stays resident in SBUF (161 KiB/partition) so HBM traffic is the
roofline minimum: read 33 rows + write 32 rows per core.

Structure: the 32 output rows are processed as 4 blocks of 8. Each block's
ratios only need energies of rows [8k, 8k+8], so block k's store phase
overlaps block k+1's load phase and the DMA engines stay saturated.

Engine split (measured per-[128,1250]-op costs): ACT does the 33 energy
squares (activation+accum_out, ~1.6us each) during the load phase, GpSimd the
32 ratio-scale multiplies (~1.5us), DVE the 32 adds (~1.75us) during the
store phase. vector.tensor_tensor_reduce is avoided - it crashes this runtime.
"""

import numpy as np

B = 256
S = 160000
P = 128
F = S // P            # 1250 samples per partition per row
N_CORES = 8
OUT_ROWS = B // N_CORES   # 32
ROWS = OUT_ROWS + 1       # +1 halo row
EPS = 1e-10
INV_N = 1.0 / S

# pipelined block sizes: small first block (fast ramp to the first output
# DMAs), small last block (short drain tail), 8-row blocks in the middle
BLOCK_SIZES = (4, 8, 8, 8, 4)
assert sum(BLOCK_SIZES) == OUT_ROWS

_cache = {}


def _build_nc():
    from contextlib import ExitStack

    import concourse.bass as bass
    import concourse.tile as tile
    from concourse import bacc, mybir

    nc = bacc.Bacc("TRN2", target_bir_lowering=False, debug=False,
                   num_devices=N_CORES)
    f32 = mybir.dt.float32
    wv = nc.declare_dram_parameter("waveforms", [ROWS, S], f32, isOutput=False)
    out = nc.declare_dram_parameter("out", [OUT_ROWS, S], f32, isOutput=True)

    in_v = wv.ap().rearrange("r (p f) -> p r f", p=P)    # [128, 33, 1250]
    out_v = out.ap().rearrange("r (p f) -> p r f", p=P)  # [128, 32, 1250]

    with tile.TileContext(nc) as tc, ExitStack() as ctx:
        data_pool = ctx.enter_context(tc.tile_pool(name="data", bufs=1))
        scr_pool = ctx.enter_context(tc.tile_pool(name="scr", bufs=1))
        outp = ctx.enter_context(tc.tile_pool(name="outp", bufs=3))
        singles = ctx.enter_context(tc.tile_pool(name="singles", bufs=1))
        psum = ctx.enter_context(tc.tile_pool(name="psum", bufs=2, space="PSUM"))

        data = data_pool.tile([P, ROWS * F], f32)
        partials = singles.tile([P, ROWS], f32)       # per-partition sum(x^2)
        inv_n_col = singles.tile([P, 1], f32)         # 1/S for the mean matmul
        ones_row = singles.tile([1, P], f32)          # broadcast matmul lhsT
        e_sb = singles.tile([1, ROWS], f32)           # mean energies
        e_bc = singles.tile([P, ROWS], f32)           # energies on all partitions
        denom = singles.tile([P, OUT_ROWS], f32)
        ratio = singles.tile([P, OUT_ROWS], f32)      # clipped mix ratios
        sq_act = scr_pool.tile([P, F], f32, tag="sq_act")

        nc.vector.memset(inv_n_col[:], INV_N)
        nc.gpsimd.memset(ones_row[:], 1.0)

        def load_rows(r0, r1, split=1):
            # in-DMAs ride GpSimd/SWDGE: gpsimd is otherwise idle, so loads
            # are never queued behind out-DMAs on Sync's in-order stream
            step = max(1, (r1 - r0 + split - 1) // split)
            for g in range(r0, r1, step):
                ge = min(g + step, r1)
                nc.gpsimd.dma_start(out=data[:, g * F:ge * F],
                                    in_=in_v[:, g:ge, :])

        def square(r):
            nc.scalar.activation(
                out=sq_act[:], in_=data[:, r * F:(r + 1) * F],
                func=mybir.ActivationFunctionType.Square,
                accum_out=partials[:, r:r + 1],
            )

        def block_ratio(lo, hi):
            # energies for rows [lo, hi] -> ratio[:, lo:hi] on all partitions
            n = hi - lo + 1
            e_ps = psum.tile([1, n], f32, tag="e")
            nc.tensor.matmul(e_ps[:], inv_n_col[:], partials[:, lo:hi + 1],
                             start=True, stop=True)
            nc.vector.tensor_copy(e_sb[:, lo:hi + 1], e_ps[:])
            bc_ps = psum.tile([P, n], f32, tag="bc")
            nc.tensor.matmul(bc_ps[:], ones_row[:], e_sb[:, lo:hi + 1],
                             start=True, stop=True)
            nc.vector.tensor_copy(e_bc[:, lo:hi + 1], bc_ps[:])
            rs = slice(lo, hi)
            nc.vector.tensor_scalar_max(denom[:, rs], e_bc[:, lo + 1:hi + 1], EPS)
            nc.vector.reciprocal(denom[:, rs], denom[:, rs])
            nc.vector.tensor_mul(denom[:, rs], e_bc[:, lo:hi], denom[:, rs])
            nc.scalar.sqrt(ratio[:, rs], denom[:, rs])
            nc.vector.tensor_scalar(
                out=ratio[:, rs], in0=ratio[:, rs], scalar1=50.0, scalar2=0.02,
                op0=mybir.AluOpType.min, op1=mybir.AluOpType.max,
            )

        def mix_row(r, on_dve):
            # out[r] = w[r] + ratio[r] * w[r+1]; adds on DVE, scales split
            # ACT/DVE to balance engine busy (ACT also owns the squares).
            # (NOT gpsimd: its tensor_scalar measured 19us/op vs ACT 1.6us)
            o = outp.tile([P, F], f32, tag="o")
            nxt = data[:, (r + 1) * F:(r + 2) * F]
            if on_dve:
                nc.vector.tensor_scalar_mul(o[:], nxt, ratio[:, r:r + 1])
            else:
                nc.scalar.mul(o[:], nxt, mul=ratio[:, r:r + 1])
            nc.vector.tensor_add(o[:], o[:], data[:, r * F:(r + 1) * F])
            nc.sync.dma_start(out=out_v[:, r, :], in_=o[:])

        # Software pipeline over blocks. block k's ratio chain is emitted at
        # the START of iteration k-1 - at that point every input (squares of
        # block k, finished during block k-2's window) is already done, so
        # the chain's ~5us latency never stalls DVE's in-order stream, and
        # ratio(k) is long since ready when mix(k) starts. Emission order
        # also keeps block k's scale-muls before block k+2's squares on ACT,
        # so out-DMAs are never starved behind a square batch.
        nb = len(BLOCK_SIZES)
        starts = [sum(BLOCK_SIZES[:i]) for i in range(nb + 1)]

        def load_and_square(k, split=1):
            # block k's not-yet-loaded rows, incl. its halo row starts[k+1]
            lo = starts[k] + (1 if k else 0)
            hi = starts[k + 1] + 1
            load_rows(lo, hi, split=split)
            for r in range(lo, hi):
                square(r)

        load_and_square(0, split=BLOCK_SIZES[0] + 1)  # per-row DMAs: ramp
        load_and_square(1)
        block_ratio(starts[0], starts[1])
        for k in range(nb):
            for i, r in enumerate(range(starts[k], starts[k + 1])):
                mix_row(r, on_dve=(i % 8 < 3))
            if k + 1 < nb:
                block_ratio(starts[k + 1], starts[k + 2])
            if k + 2 < nb:
                load_and_square(k + 2)

    nc.compile()
    return nc


def _get_nc():
    if "nc" not in _cache:
        _cache["nc"] = _build_nc()
    return _cache["nc"]


def _shard_inputs(waveforms):
    in_maps = []
    for c in range(N_CORES):
        rows = (np.arange(c * OUT_ROWS, c * OUT_ROWS + ROWS)) % B
        in_maps.append({"waveforms": np.ascontiguousarray(waveforms[rows])})
    return in_maps


def kernel(waveforms):
    from concourse.bass_utils import run_bass_kernel_spmd

    waveforms = np.asarray(waveforms, dtype=np.float32)
    nc = _get_nc()
    in_maps = _shard_inputs(waveforms)
    res = run_bass_kernel_spmd(nc, in_maps, list(range(N_CORES)))
    mixtures = np.concatenate(
        [res.results[c]["out"] for c in range(N_CORES)], axis=0
    )
    return mixtures, waveforms


# revision 15
# speedup vs baseline: 1.1977x; 1.1565x over previous
"""AnchorSegmentMixer Trainium2 kernel (8 NeuronCores, batch-sharded).

reference:
    energy[n] = mean(w[n]**2)                       # [B]
    ratio[n]  = clip(sqrt(energy[n]/max(energy[n+1 mod B], 1e-10)), 0.02, 50)
    mixtures  = w + ratio[:, None] * roll(w, -1, axis=0)
    returns (mixtures, targets=w)

Sharding: pure data parallel over the batch axis. Core c receives rows
[32c, 32c+32] (33 rows: 32 output rows + 1 circular halo row), computes all 33
row energies locally, and emits its 32 mixture rows. No collectives needed.

On-chip layout: each 160000-sample row is spread over the 128 SBUF partitions
as [128, 1250] (partition p holds samples [1250p, 1250(p+1))), and the whole
33-row shard stays resident in SBUF (161 KiB/partition) so HBM traffic is the
roofline minimum: read 33 rows + write 32 rows per core.

Structure: the 32 output rows are processed as 4 blocks of 8. Each block's
ratios only need energies of rows [8k, 8k+8], so block k's store phase
overlaps block k+1's load phase and the DMA engines stay saturated.

Engine split (measured per-[128,1250]-op costs): ACT does the 33 energy
squares (activation+accum_out, ~1.6us each) during the load phase, GpSimd the
32 ratio-scale multiplies (~1.5us), DVE the 32 adds (~1.75us) during the
store phase. vector.tensor_tensor_reduce is avoided - it crashes this runtime.
"""

import numpy as np

B = 256
S = 160000
P = 128
F = S // P            # 1250 samples per partition per row
N_CORES = 8
OUT_ROWS = B // N_CORES   # 32
ROWS = OUT_ROWS + 1       # +1 halo row
EPS = 1e-10
INV_N = 1.0 / S

# pipelined block sizes: small first block (fast ramp to the first output
# DMAs), small last block (short drain tail), 8-row blocks in the middle
BLOCK_SIZES = (4, 8, 8, 8, 4)
assert sum(BLOCK_SIZES) == OUT_ROWS

_cache = {}


def _build_nc():
    from contextlib import ExitStack

    import concourse.bass as bass
    import concourse.tile as tile
    from concourse import bacc, mybir

    nc = bacc.Bacc("TRN2", target_bir_lowering=False, debug=False,
                   num_devices=N_CORES)
    f32 = mybir.dt.float32
    wv = nc.declare_dram_parameter("waveforms", [ROWS, S], f32, isOutput=False)
    out = nc.declare_dram_parameter("out", [OUT_ROWS, S], f32, isOutput=True)

    in_v = wv.ap().rearrange("r (p f) -> p r f", p=P)    # [128, 33, 1250]
    out_v = out.ap().rearrange("r (p f) -> p r f", p=P)  # [128, 32, 1250]

    with tile.TileContext(nc) as tc, ExitStack() as ctx:
        data_pool = ctx.enter_context(tc.tile_pool(name="data", bufs=1))
        scr_pool = ctx.enter_context(tc.tile_pool(name="scr", bufs=1))
        outp = ctx.enter_context(tc.tile_pool(name="outp", bufs=3))
        singles = ctx.enter_context(tc.tile_pool(name="singles", bufs=1))
        psum = ctx.enter_context(tc.tile_pool(name="psum", bufs=2, space="PSUM"))

        data = data_pool.tile([P, ROWS * F], f32)
        partials = singles.tile([P, ROWS], f32)       # per-partition sum(x^2)
        inv_n_col = singles.tile([P, 1], f32)         # 1/S for the mean matmul
        ones_row = singles.tile([1, P], f32)          # broadcast matmul lhsT
        e_sb = singles.tile([1, ROWS], f32)           # mean energies
        e_bc = singles.tile([P, ROWS], f32)           # energies on all partitions
        denom = singles.tile([P, OUT_ROWS], f32)
        ratio = singles.tile([P, OUT_ROWS], f32)      # clipped mix ratios
        sq_act = scr_pool.tile([P, F], f32, tag="sq_act")

        nc.vector.memset(inv_n_col[:], INV_N)
        nc.gpsimd.memset(ones_row[:], 1.0)

        def load_rows(r0, r1, split=1):
            # in-DMAs ride GpSimd/SWDGE: gpsimd is otherwise idle, so loads
            # are never queued behind out-DMAs on Sync's in-order stream
            step = max(1, (r1 - r0 + split - 1) // split)
            for g in range(r0, r1, step):
                ge = min(g + step, r1)
                nc.gpsimd.dma_start(out=data[:, g * F:ge * F],
                                    in_=in_v[:, g:ge, :])

        def square(r):
            nc.scalar.activation(
                out=sq_act[:], in_=data[:, r * F:(r + 1) * F],
                func=mybir.ActivationFunctionType.Square,
                accum_out=partials[:, r:r + 1],
            )

        def block_ratio(lo, hi):
            # energies for rows [lo, hi] -> ratio[:, lo:hi] on all partitions
            n = hi - lo + 1
            e_ps = psum.tile([1, n], f32, tag="e")
            nc.tensor.matmul(e_ps[:], inv_n_col[:], partials[:, lo:hi + 1],
                             start=True, stop=True)
            nc.vector.tensor_copy(e_sb[:, lo:hi + 1], e_ps[:])
            bc_ps = psum.tile([P, n], f32, tag="bc")
            nc.tensor.matmul(bc_ps[:], ones_row[:], e_sb[:, lo:hi + 1],
                             start=True, stop=True)
            nc.vector.tensor_copy(e_bc[:, lo:hi + 1], bc_ps[:])
            rs = slice(lo, hi)
            nc.vector.tensor_scalar_max(denom[:, rs], e_bc[:, lo + 1:hi + 1], EPS)
            nc.vector.reciprocal(denom[:, rs], denom[:, rs])
            nc.vector.tensor_mul(denom[:, rs], e_bc[:, lo:hi], denom[:, rs])
            nc.scalar.sqrt(ratio[:, rs], denom[:, rs])
            nc.vector.tensor_scalar(
                out=ratio[:, rs], in0=ratio[:, rs], scalar1=50.0, scalar2=0.02,
                op0=mybir.AluOpType.min, op1=mybir.AluOpType.max,
            )

        def mix_row(r):
            # out[r] = w[r] + ratio[r]*w[r+1] in ONE custom-DVE op
            # (affine_then_add) - one engine owns the whole store-side
            # compute, so ACT (squares) and DVE (mix) never contend.
            o = outp.tile([P, F], f32, tag="o")
            nc.vector.affine_then_add(
                out=o[:], in0=data[:, (r + 1) * F:(r + 2) * F],
                in1=data[:, r * F:(r + 1) * F],
                scale=ratio[:, r:r + 1], bias=0.0,
            )
            nc.sync.dma_start(out=out_v[:, r, :], in_=o[:])

        # Software pipeline over blocks; one-block lookahead on the loads.
        # Tile's scheduler reorders within the dataflow DAG, but with the
        # fused mix op each phase has a single owner engine (ACT: squares,
        # DVE: mix, PE: ratio matmuls) so ordering hazards are gone.
        nb = len(BLOCK_SIZES)
        starts = [sum(BLOCK_SIZES[:i]) for i in range(nb + 1)]

        def load_and_square(k, split=1):
            # block k's not-yet-loaded rows, incl. its halo row starts[k+1]
            lo = starts[k] + (1 if k else 0)
            hi = starts[k + 1] + 1
            load_rows(lo, hi, split=split)
            for r in range(lo, hi):
                square(r)

        load_and_square(0, split=BLOCK_SIZES[0] + 1)  # per-row DMAs: ramp
        for k in range(nb):
            if k + 1 < nb:
                load_and_square(k + 1)
            block_ratio(starts[k], starts[k + 1])
            for r in range(starts[k], starts[k + 1]):
                mix_row(r)

    nc.compile()
    return nc


def _get_nc():
    if "nc" not in _cache:
        _cache["nc"] = _build_nc()
    return _cache["nc"]


def _shard_inputs(waveforms):
    in_maps = []
    for c in range(N_CORES):
        rows = (np.arange(c * OUT_ROWS, c * OUT_ROWS + ROWS)) % B
        in_maps.append({"waveforms": np.ascontiguousarray(waveforms[rows])})
    return in_maps


def kernel(waveforms):
    from concourse.bass_utils import run_bass_kernel_spmd

    waveforms = np.asarray(waveforms, dtype=np.float32)
    nc = _get_nc()
    in_maps = _shard_inputs(waveforms)
    res = run_bass_kernel_spmd(nc, in_maps, list(range(N_CORES)))
    mixtures = np.concatenate(
        [res.results[c]["out"] for c in range(N_CORES)], axis=0
    )
    return mixtures, waveforms


# revision 20
# speedup vs baseline: 1.2054x; 1.0065x over previous
"""AnchorSegmentMixer Trainium2 kernel (8 NeuronCores, batch-sharded).

reference:
    energy[n] = mean(w[n]**2)                       # [B]
    ratio[n]  = clip(sqrt(energy[n]/max(energy[n+1 mod B], 1e-10)), 0.02, 50)
    mixtures  = w + ratio[:, None] * roll(w, -1, axis=0)
    returns (mixtures, targets=w)

Sharding: pure data parallel over the batch axis. Core c receives rows
[32c, 32c+32] (33 rows: 32 output rows + 1 circular halo row), computes all 33
row energies locally, and emits its 32 mixture rows. No collectives needed.

On-chip layout: each 160000-sample row is spread over the 128 SBUF partitions
as [128, 1250] (partition p holds samples [1250p, 1250(p+1))), and the whole
33-row shard stays resident in SBUF (161 KiB/partition) so HBM traffic is the
roofline minimum: read 33 rows + write 32 rows per core.

Structure: the 32 output rows are processed as 4 blocks of 8. Each block's
ratios only need energies of rows [8k, 8k+8], so block k's store phase
overlaps block k+1's load phase and the DMA engines stay saturated.

Engine split (measured per-[128,1250]-op costs): ACT does the 33 energy
squares (activation+accum_out, ~1.6us each) during the load phase, GpSimd the
32 ratio-scale multiplies (~1.5us), DVE the 32 adds (~1.75us) during the
store phase. vector.tensor_tensor_reduce is avoided - it crashes this runtime.
"""

import numpy as np

B = 256
S = 160000
P = 128
F = S // P            # 1250 samples per partition per row
N_CORES = 8
OUT_ROWS = B // N_CORES   # 32
ROWS = OUT_ROWS + 1       # +1 halo row
EPS = 1e-10
INV_N = 1.0 / S

# pipelined block sizes: small first block (fast ramp to the first output
# DMAs), small last block (short drain tail), 8-row blocks in the middle
BLOCK_SIZES = (4, 8, 8, 8, 4)
assert sum(BLOCK_SIZES) == OUT_ROWS

_cache = {}


def _build_nc():
    from contextlib import ExitStack

    import concourse.bass as bass
    import concourse.tile as tile
    from concourse import bacc, mybir

    nc = bacc.Bacc("TRN2", target_bir_lowering=False, debug=False,
                   num_devices=N_CORES)
    f32 = mybir.dt.float32
    wv = nc.declare_dram_parameter("waveforms", [ROWS, S], f32, isOutput=False)
    out = nc.declare_dram_parameter("out", [OUT_ROWS, S], f32, isOutput=True)

    in_v = wv.ap().rearrange("r (p f) -> p r f", p=P)    # [128, 33, 1250]
    out_v = out.ap().rearrange("r (p f) -> p r f", p=P)  # [128, 32, 1250]

    with tile.TileContext(nc) as tc, ExitStack() as ctx:
        data_pool = ctx.enter_context(tc.tile_pool(name="data", bufs=1))
        scr_pool = ctx.enter_context(tc.tile_pool(name="scr", bufs=1))
        outp = ctx.enter_context(tc.tile_pool(name="outp", bufs=4))
        singles = ctx.enter_context(tc.tile_pool(name="singles", bufs=1))
        psum = ctx.enter_context(tc.tile_pool(name="psum", bufs=2, space="PSUM"))

        data = data_pool.tile([P, ROWS * F], f32)
        partials = singles.tile([P, ROWS], f32)       # per-partition sum(x^2)
        inv_n_col = singles.tile([P, 1], f32)         # 1/S for the mean matmul
        ones_row = singles.tile([1, P], f32)          # broadcast matmul lhsT
        e_sb = singles.tile([1, ROWS], f32)           # mean energies
        denom = singles.tile([1, OUT_ROWS], f32)      # chain scratch [1,n]
        rat1 = singles.tile([1, OUT_ROWS], f32)       # clipped ratios [1,n]
        ratio = singles.tile([P, OUT_ROWS], f32)      # broadcast mix ratios
        sq_act = scr_pool.tile([P, F], f32, tag="sq_act")

        nc.vector.memset(inv_n_col[:], INV_N)
        nc.gpsimd.memset(ones_row[:], 1.0)

        def load_rows(r0, r1, split=1, engine=None):
            # in-DMAs ride GpSimd/SWDGE: gpsimd is otherwise idle, so loads
            # are never queued behind out-DMAs on Sync's in-order stream.
            # (The first block's loads go on Sync instead: at t=0 Sync has no
            # out-DMAs yet, and SWDGE inter-DMA drains would slow the ramp.)
            eng = engine or nc.gpsimd
            step = max(1, (r1 - r0 + split - 1) // split)
            for g in range(r0, r1, step):
                ge = min(g + step, r1)
                eng.dma_start(out=data[:, g * F:ge * F],
                              in_=in_v[:, g:ge, :])

        def square(r):
            nc.scalar.activation(
                out=sq_act[:], in_=data[:, r * F:(r + 1) * F],
                func=mybir.ActivationFunctionType.Square,
                accum_out=partials[:, r:r + 1],
            )

        def block_ratio(lo, hi):
            # energies for rows [lo, hi] -> ratio[:, lo:hi] on all
            # partitions. Everything except the final broadcast runs on tiny
            # [1, n] vectors; clip is applied to the ratio SQUARED (bounds
            # 0.02^2 / 50^2) so the single sqrt comes last and the chain has
            # only one ACT<->DVE hop before the broadcast matmul.
            n = hi - lo + 1
            e_ps = psum.tile([1, n], f32, tag="e")
            nc.tensor.matmul(e_ps[:], inv_n_col[:], partials[:, lo:hi + 1],
                             start=True, stop=True)
            nc.vector.tensor_copy(e_sb[:, lo:hi + 1], e_ps[:])
            q = denom[:1, lo:hi]
            nc.vector.tensor_scalar_max(q, e_sb[:, lo + 1:hi + 1], EPS)
            nc.vector.reciprocal(q, q)
            nc.vector.tensor_mul(q, e_sb[:, lo:hi], q)
            nc.vector.tensor_scalar(
                out=q, in0=q, scalar1=2500.0, scalar2=0.0004,
                op0=mybir.AluOpType.min, op1=mybir.AluOpType.max,
            )
            nc.scalar.sqrt(rat1[:, lo:hi], q)
            bc_ps = psum.tile([P, n - 1], f32, tag="bc")
            nc.tensor.matmul(bc_ps[:], ones_row[:], rat1[:, lo:hi],
                             start=True, stop=True)
            nc.vector.tensor_copy(ratio[:, lo:hi], bc_ps[:])

        def mix_row(r):
            # out[r] = w[r] + ratio[r]*w[r+1] in ONE custom-DVE op
            # (affine_then_add) - one engine owns the whole store-side
            # compute, so ACT (squares) and DVE (mix) never contend.
            o = outp.tile([P, F], f32, tag="o")
            nc.vector.affine_then_add(
                out=o[:], in0=data[:, (r + 1) * F:(r + 2) * F],
                in1=data[:, r * F:(r + 1) * F],
                scale=ratio[:, r:r + 1], bias=0.0,
            )
            nc.sync.dma_start(out=out_v[:, r, :], in_=o[:])

        # Software pipeline over blocks; one-block lookahead on the loads.
        # Tile's scheduler reorders within the dataflow DAG, but with the
        # fused mix op each phase has a single owner engine (ACT: squares,
        # DVE: mix, PE: ratio matmuls) so ordering hazards are gone.
        nb = len(BLOCK_SIZES)
        starts = [sum(BLOCK_SIZES[:i]) for i in range(nb + 1)]

        def load_and_square(k, split=1):
            # block k's not-yet-loaded rows, incl. its halo row starts[k+1]
            lo = starts[k] + (1 if k else 0)
            hi = starts[k + 1] + 1
            load_rows(lo, hi, split=split,
                      engine=nc.sync if k == 0 else None)
            for r in range(lo, hi):
                square(r)

        load_and_square(0, split=BLOCK_SIZES[0] + 1)  # per-row DMAs: ramp
        for k in range(nb):
            if k + 1 < nb:
                load_and_square(k + 1)
            block_ratio(starts[k], starts[k + 1])
            for r in range(starts[k], starts[k + 1]):
                mix_row(r)

    nc.compile()
    return nc


def _get_nc():
    if "nc" not in _cache:
        _cache["nc"] = _build_nc()
    return _cache["nc"]


def _shard_inputs(waveforms):
    in_maps = []
    for c in range(N_CORES):
        rows = (np.arange(c * OUT_ROWS, c * OUT_ROWS + ROWS)) % B
        in_maps.append({"waveforms": np.ascontiguousarray(waveforms[rows])})
    return in_maps


def kernel(waveforms):
    from concourse.bass_utils import run_bass_kernel_spmd

    waveforms = np.asarray(waveforms, dtype=np.float32)
    nc = _get_nc()
    in_maps = _shard_inputs(waveforms)
    res = run_bass_kernel_spmd(nc, in_maps, list(range(N_CORES)))
    mixtures = np.concatenate(
        [res.results[c]["out"] for c in range(N_CORES)], axis=0
    )
    return mixtures, waveforms
